# revision 10
# baseline (speedup 1.0000x reference)
"""CrossProductLayer kernel for Trainium2 (Bass/Tile), 8-core data parallel.

out[b, :] = concat(x[b]**2, x[b], 0.5 * x[b,i]*x[b,j] for i<j) * w

Full inputs:  x [16384, 128] f32, w [8384] f32.
Full output:  [16384, 8384] f32.

Sharding: pure data parallel on the batch dim — each of the 8 cores gets
2048 rows of x; w (pre-scaled by the 0.5 pair factor and pre-broadcast to
[128, 8384] on host) is replicated. Forward only, no collectives.

Per-core device kernel. The core's 2048 rows = 16 row-tiles of 128
(partition dim). Work is organized in 8 units = (4 tile-groups of G=4
row-tiles) x (2 column halves of 4192). A unit's SBUF tile is
[128, 4 x 4192] (67 KB/partition, bufs=2): storing it to HBM writes
16.8 KB contiguous rows, which sustains ~428 GB/s (narrow column chunks
measured as low as 198 GB/s — row length is the DMA efficiency lever).

Within a unit, each pair block i (out[:, blk] = x[:,i] * x[:,i+1:]) is a
single grouped op over [128, 4, w] APs, split across engines by a
calibrated cost model:
  - ScalarE:  widest blocks (per-tile activation with per-partition
              scale; 445-cycle op overhead amortizes over wide blocks),
              plus the squares block. Singles come in by DMA.
  - GpSimdE:  middle blocks (grouped broadcast tensor_tensor) + a share
              of the *w pass.
  - VectorE:  narrow tail + the rest of the *w pass.
VectorE issues only tensor_tensor ops (1-port) so GpSimdE never contends
for the shared SBUF port. The *w pass is in-place per unit, split
column-wise between VectorE/GpSimdE to balance their loads.
"""

import numpy as np

B = 16384
NI = 128
NF = NI + NI + (NI * (NI - 1)) // 2  # 8384
NCORES = 8
ROWS = B // NCORES  # 2048
TILE_P = 128
TILES = ROWS // TILE_P  # 16
PAIRS_OFF = 2 * NI  # 256

G = 4  # row-tiles per group
NG = TILES // G  # 4 groups
HALF = NF // 2  # 4192

WIDTHS = [NI - 1 - i for i in range(NI - 1)]
STARTS = []
_off = PAIRS_OFF
for _w in WIDTHS:
    STARTS.append(_off)
    _off += _w
assert _off == NF

# calibrated per-op cost model (ns) for engine balancing, per unit (G=4)
ACT_OP = lambda w: (445.0 + w) / 1.2  # one per-tile op
DVE_OP = lambda w: (150.0 + G * (w + 32)) / 0.96  # grouped op
GP_OP = lambda w: 250.0 + G * w * 2.4 / 1.2  # grouped op
DVE_WCOL = G * 1.05 / 0.96  # *w pass per col
GP_WCOL = G * 2.4 / 1.2


def _pieces(h):
    """Blocks (sub-ranges) whose output cols intersect half h."""
    lo, hi = h * HALF, (h + 1) * HALF
    out = []
    for i in range(NI - 1):
        s, w = STARTS[i], WIDTHS[i]
        a, b = max(s, lo), min(s + w, hi)
        if a < b:
            out.append((i, a, b - a))  # block, out col start, piece width
    return out


def _plan():
    """Pick stripe boundaries A (ACT) and B (GpSimd) by global balance."""
    best = None
    for A in range(10, 60):
        act = NG * (445 + NI * G) / 1.2  # grouped squares, 4 ops
        act += sum(TILES * ACT_OP(WIDTHS[i]) for i in range(A))
        for Bb in range(A, NI - 1):
            gp = sum(NG * GP_OP(WIDTHS[i]) for i in range(A, Bb))
            dve = sum(NG * DVE_OP(WIDTHS[i]) for i in range(Bb, NI - 1))
            # *w pass: NF cols total (head included), split d vs g:
            # dve + NG*DVE_WCOL*wd = gp + NG*GP_WCOL*(NF - wd)
            wd = (gp - dve + GP_WCOL * NG * NF) / (
                NG * (DVE_WCOL + GP_WCOL)
            )
            wd = min(max(wd, 0.0), float(NF))
            d_tot = dve + DVE_WCOL * NG * wd
            g_tot = gp + GP_WCOL * NG * (NF - wd)
            mk = max(act, d_tot, g_tot)
            if best is None or mk < best[0]:
                best = (mk, A, Bb, act, d_tot, g_tot)
    return best


_BEST = _plan()
STRIPE_A, STRIPE_B = _BEST[1], _BEST[2]

_CACHE = {}


def _build_nc():
    from concourse import bacc
    import concourse.mybir as mybir
    from concourse.tile import TileContext

    f32 = mybir.dt.float32
    nc = bacc.Bacc(
        "TRN2",
        target_bir_lowering=False,
        debug=False,
        num_devices=NCORES,
    )
    x_d = nc.dram_tensor("x", [ROWS, NI], f32, kind="ExternalInput")
    w_d = nc.dram_tensor("w", [NI, NF], f32, kind="ExternalInput")
    o_d = nc.dram_tensor("out", [ROWS, NF], f32, kind="ExternalOutput")

    x_hbm3 = x_d.rearrange("(t p) c -> p t c", t=TILES)  # [128, 16, 128]
    o_hbm3 = o_d.rearrange("(t p) c -> p t c", t=TILES)  # [128, 16, 8384]

    halves = [_pieces(0), _pieces(1)]

    with TileContext(nc) as tc:
        with (
            tc.tile_pool(name="xp", bufs=1) as xp,
            tc.tile_pool(name="wp", bufs=1) as wp,
            tc.tile_pool(name="pp", bufs=2) as pp,
        ):
            x_all = xp.tile([TILE_P, TILES * NI], f32)
            x3 = x_all[:].rearrange("p (t c) -> p t c", t=TILES)
            nc.sync.dma_start(out=x3, in_=x_hbm3)
            w_t = wp.tile([NI, NF], f32)
            nc.sync.dma_start(out=w_t[:], in_=w_d[:])

            for g in range(NG):
                t0, t1 = g * G, (g + 1) * G
                xg = x3[:, t0:t1]  # [128, 4, 128]
                for h in range(2):
                    lo = h * HALF
                    pan = pp.tile(
                        [TILE_P, G * HALF], f32, name=f"pan{g}{h}", tag="pan"
                    )
                    pan3 = pan[:].rearrange("p (t c) -> p t c", t=G)
                    load = {"A": 0.0, "G": 0.0, "D": 0.0}
                    if h == 0:
                        nc.sync.dma_start(
                            out=pan3[:, :, NI : 2 * NI],
                            in_=x_hbm3[:, t0:t1],
                        )
                        nc.scalar.square(pan3[:, :, 0:NI], xg)
                        load["A"] += (445 + NI * G) / 1.2
                    for i, cs, w in halves[h]:
                        c = cs - lo
                        j0 = i + 1 + (cs - STARTS[i])
                        src = x3[:, t0:t1, j0 : j0 + w]
                        dst = pan3[:, :, c : c + w]
                        if i < STRIPE_A:
                            for t in range(G):
                                nc.scalar.mul(
                                    dst[:, t],
                                    src[:, t],
                                    x3[:, t0 + t, i : i + 1],
                                )
                            load["A"] += G * ACT_OP(w)
                        else:
                            bc = x3[:, t0:t1, i : i + 1].broadcast_to(
                                [TILE_P, G, w]
                            )
                            if i < STRIPE_B:
                                nc.gpsimd.tensor_mul(dst, bc, src)
                                load["G"] += GP_OP(w)
                            else:
                                nc.vector.tensor_mul(dst, bc, src)
                                load["D"] += DVE_OP(w)
                    # in-place *w pass, split between VectorE and GpSimd
                    wd = (load["G"] - load["D"] + GP_WCOL * HALF) / (
                        DVE_WCOL + GP_WCOL
                    )
                    wd = int(np.clip(round(wd), 0, HALF))
                    wsl = w_t[:, None, lo : lo + HALF]
                    if wd > 0:
                        nc.vector.tensor_mul(
                            pan3[:, :, 0:wd],
                            pan3[:, :, 0:wd],
                            wsl[:, :, 0:wd].broadcast_to([TILE_P, G, wd]),
                        )
                    if wd < HALF:
                        nc.gpsimd.tensor_mul(
                            pan3[:, :, wd:HALF],
                            pan3[:, :, wd:HALF],
                            wsl[:, :, wd:HALF].broadcast_to(
                                [TILE_P, G, HALF - wd]
                            ),
                        )
                    nc.sync.dma_start(
                        out=o_hbm3[:, t0:t1, lo : lo + HALF], in_=pan3
                    )
    nc.compile()
    return nc


def _get_nc():
    if "nc" not in _CACHE:
        _CACHE["nc"] = _build_nc()
    return _CACHE["nc"]


def _prep_in_maps(x, w):
    x = np.ascontiguousarray(np.asarray(x, dtype=np.float32))
    w = np.asarray(w, dtype=np.float32)
    w_scaled = w.copy()
    w_scaled[PAIRS_OFF:] *= np.float32(0.5)
    w_b = np.ascontiguousarray(np.broadcast_to(w_scaled[None, :], (NI, NF)))
    return [
        {"x": np.ascontiguousarray(x[c * ROWS : (c + 1) * ROWS]), "w": w_b}
        for c in range(NCORES)
    ]


def _run(x, w, trace=False, tmpdir=None):
    from concourse.bass_utils import run_bass_kernel_spmd

    nc = _get_nc()
    in_maps = _prep_in_maps(x, w)
    res = run_bass_kernel_spmd(
        nc, in_maps, list(range(NCORES)), trace=trace, tmpdir=tmpdir
    )
    out = np.concatenate([res.results[c]["out"] for c in range(NCORES)], axis=0)
    return out, res


def kernel(**inputs):
    out, _ = _run(inputs["x"], inputs["w"])
    return out


if __name__ == "__main__":
    mk, A, Bb, act, d_tot, g_tot = _BEST
    print(f"A={A} B={Bb}")
    print(
        f"pred ACT={act/1e3:.0f}us DVE={d_tot/1e3:.0f}us GP={g_tot/1e3:.0f}us"
        f" makespan={mk/1e3:.0f}us"
    )
